# revision 1
# baseline (speedup 1.0000x reference)
"""CRF NLL (mean) loss kernel for Trainium2, 8 NeuronCores.

Strategy (hardcoded for B=256, S=512, T=64):
  - Data-parallel over batch: 32 sequences per core.
  - Denominator (log-partition) on device: exp-space forward scan
      alphaT_{s} = (expM.T @ alphaT_{s-1}) * expEmT_s        [T=64 part, B=32 free]
    with periodic renormalization (column sums via ones-matmul) to stay in
    f32 range; log of the renorm constants accumulates into the result.
  - Numerator (gold path score) on host in numpy (gathers; ~0.3% of FLOPs).
  - Final mean on host.
"""

import os
import sys

import numpy as np

sys.path.insert(0, "/opt/trn_rl_repo")

B, S, T = 256, 512, 64
NCORES = 8
BL = B // NCORES  # 32 sequences per core
CHUNK = 64        # scan steps per ACT-exp chunk
RENORM = 16       # renormalize every RENORM steps

_CACHE = {}


def _build_nc(S=S, CHUNK=CHUNK, RENORM=RENORM, split_waits=True):
    # Device kernel per core: exp-space forward scan over S steps.
    #   em_all [BL, S*T] stays resident in SBUF (4 MB shard).
    #   chunked ACT exp -> per-step DVE transposes -> chain:
    #       psum = expM.T @ alphaT (PE) ; alphaT = psum * eT_s (DVE)
    #   every RENORM steps: colsum via ones-matmul, stash c into `strip`,
    #   rescale alpha by 1/c (outer-product matmul + mul).
    #   Output: strip [1, (NR+1)*BL] of renorm constants + final Z;
    #   host computes denom = sum(log(strip)) per sequence.
    import concourse.bass as bass
    import concourse.mybir as mybir
    from concourse import tile

    AF = mybir.ActivationFunctionType
    f32 = mybir.dt.float32
    NR = S // RENORM  # renorm count (last one folds into final Z slot too)

    nc = bass.Bass()
    em_d = nc.dram_tensor("em", [BL, S * T], f32, kind="ExternalInput")
    expM_d = nc.dram_tensor("expM", [T, T], f32, kind="ExternalInput")
    startT_d = nc.dram_tensor("startT", [T, 1], f32, kind="ExternalInput")
    expEnd_d = nc.dram_tensor("expEnd", [T, 1], f32, kind="ExternalInput")
    cs_d = nc.dram_tensor("cs", [1, (NR + 1) * BL], f32, kind="ExternalOutput")

    with tile.TileContext(nc) as tc:
        with (
            tc.tile_pool(name="consts", bufs=1) as consts,
            tc.tile_pool(name="embuf", bufs=1) as emp,
            tc.tile_pool(name="exp", bufs=2) as expp,
            tc.tile_pool(name="et", bufs=8) as etp,
            tc.tile_pool(name="alpha", bufs=4) as ap_,
            tc.tile_pool(name="small", bufs=4) as smallp,
            tc.tile_pool(name="psum", bufs=2, space="PSUM") as psp,
            tc.tile_pool(name="psum_small", bufs=2, space="PSUM") as pss,
        ):
            expM_raw = consts.tile([T, T], f32)
            startT_raw = consts.tile([T, 1], f32)
            expEnd_raw = consts.tile([T, 1], f32)
            expM = consts.tile([T, T], f32)
            startT = consts.tile([T, 1], f32)
            expEnd = consts.tile([T, 1], f32)
            onesT = consts.tile([T, 1], f32)
            ones1 = consts.tile([1, T], f32)
            strip = consts.tile([1, NR + 1, BL], f32)

            nc.sync.dma_start(expM_raw[:], expM_d[:])
            nc.sync.dma_start(startT_raw[:], startT_d[:])
            nc.sync.dma_start(expEnd_raw[:], expEnd_d[:])
            # Funnel const DMAs through one DVE touch each so downstream
            # consumers wait only on the DVE semaphore (walrus rejects >1
            # sync-wait on compute instructions; see _split_multi_waits).
            nc.vector.tensor_copy(expM[:], expM_raw[:])
            nc.vector.tensor_copy(startT[:], startT_raw[:])
            nc.vector.tensor_copy(expEnd[:], expEnd_raw[:])
            nc.vector.memset(onesT[:], 1.0)
            nc.vector.memset(ones1[:], 1.0)

            # Whole emissions shard resident in SBUF: [32 part, 128KB/part].
            em_all = emp.tile([BL, S * T], f32)
            NDMA = 4
            seg = S * T // NDMA
            for q in range(NDMA):
                nc.sync.dma_start(em_all[:, q * seg : (q + 1) * seg],
                                  em_d[:, q * seg : (q + 1) * seg])

            alpha = None
            for c in range(S // CHUNK):
                s0 = c * CHUNK
                E = expp.tile([BL, CHUNK * T], f32, tag="exp")
                nc.scalar.activation(
                    E[:], em_all[:, s0 * T : (s0 + CHUNK) * T], AF.Exp)
                for j in range(CHUNK):
                    s = s0 + j
                    if s == 0:
                        # alpha0 = exp(em_0 + start): transpose raw, ACT exp
                        # with per-partition bias.
                        eTr = etp.tile([T, BL], f32, tag="et")
                        nc.vector.transpose(
                            eTr[0:32, :], em_all[:, 0:T][:, 0:32])
                        nc.vector.transpose(
                            eTr[32:64, :], em_all[:, 0:T][:, 32:64])
                        a0 = ap_.tile([T, BL], f32, tag="alpha")
                        nc.scalar.activation(a0[:], eTr[:], AF.Exp,
                                             bias=startT[:])
                        alpha = a0
                    else:
                        eT = etp.tile([T, BL], f32, tag="et")
                        nc.vector.transpose(
                            eT[0:32, :], E[:, j * T : j * T + 32])
                        nc.vector.transpose(
                            eT[32:64, :], E[:, j * T + 32 : (j + 1) * T])
                        ps = psp.tile([T, BL], f32, tag="ps")
                        nc.tensor.matmul(ps[:], expM[:], alpha[:])
                        anew = ap_.tile([T, BL], f32, tag="alpha")
                        nc.vector.tensor_mul(anew[:], ps[:], eT[:])
                        alpha = anew
                    if s % RENORM == RENORM - 1 and s != S - 1:
                        r = s // RENORM
                        csum = pss.tile([1, BL], f32, tag="csum")
                        nc.tensor.matmul(csum[:], onesT[:], alpha[:])
                        nc.vector.tensor_copy(strip[:, r, :], csum[:])
                        rec = smallp.tile([1, BL], f32, tag="rec")
                        nc.vector.reciprocal(rec[:], csum[:])
                        bc = psp.tile([T, BL], f32, tag="bc")
                        nc.tensor.matmul(bc[:], ones1[:], rec[:])
                        asc = ap_.tile([T, BL], f32, tag="alpha")
                        nc.vector.tensor_mul(asc[:], alpha[:], bc[:])
                        alpha = asc

            # Final: Z = sum_j alpha[j,b] * expEnd[j]; last renorm slot unused
            # (s=S-1 renorm skipped; Z absorbs it).
            afin = ap_.tile([T, BL], f32, tag="alpha")
            nc.vector.tensor_scalar_mul(afin[:], alpha[:], expEnd[:])
            z = pss.tile([1, BL], f32, tag="csum")
            nc.tensor.matmul(z[:], onesT[:], afin[:])
            nc.vector.tensor_copy(strip[:, NR - 1, :], z[:])
            nc.vector.memset(strip[:, NR, :], 1.0)
            nc.sync.dma_start(cs_d[:], strip[:])

    if split_waits:
        _split_multi_waits(nc)
    return nc


def _split_multi_waits(nc):
    # This toolchain's walrus rejects >1 sync-wait command per instruction
    # ("Too many sync wait commands"). Hoist all but the last wait of any
    # multi-wait instruction onto same-engine NoOps inserted just before it.
    import concourse.mybir as mybir

    for f in nc.m.functions:
        for bb in f.blocks:
            il = bb.instructions
            i = 0
            while i < len(il):
                inst = il[i]
                si = getattr(inst, "sync_info", None)
                if si is not None and len(si.on_wait) > 1:
                    waits = list(si.on_wait)
                    for k, w in enumerate(waits[:-1]):
                        nop = mybir.InstNoOp(
                            name=f"{inst.name}-w{k}", ins=[], outs=[])
                        nop.engine = inst.engine
                        nop.sync_info = mybir.SyncInfo(
                            on_wait=[w], on_update=[])
                        il.insert(i, nop)
                        i += 1
                    inst.sync_info = mybir.SyncInfo(
                        on_wait=[waits[-1]], on_update=list(si.on_update))
                i += 1


def _numerator(emissions, tags, mask, start_transitions, end_transitions, transitions):
    # Gold-path score per sequence, f64 accumulation on host.
    tg = tags.astype(np.int64)
    em = emissions.astype(np.float64)
    maskf = mask.astype(np.float64)
    b_idx = np.arange(B)
    emit = np.take_along_axis(em, tg[:, :, None], axis=2)[..., 0]      # [B, S]
    trans_sc = transitions.astype(np.float64)[tg[:, :-1], tg[:, 1:]]   # [B, S-1]
    score = start_transitions.astype(np.float64)[tg[:, 0]] + emit[:, 0]
    score = score + np.sum((trans_sc + emit[:, 1:]) * maskf[:, 1:], axis=1)
    seq_ends = np.sum(mask != 0, axis=1).astype(np.int64) - 1
    last_tags = tg[b_idx, seq_ends]
    score = score + end_transitions.astype(np.float64)[last_tags]
    return score  # [B] f64


def _denominator_host(emissions, mask, start_transitions, end_transitions, transitions):
    # General-mask fallback (never hit for the spec'd all-ones mask): scaled
    # exp-space forward scan in f64 on host.
    em = emissions.astype(np.float64)
    Mx = np.exp(transitions.astype(np.float64))
    alpha = np.exp(start_transitions.astype(np.float64)[None, :] + em[:, 0, :])
    logz = np.zeros(B)
    for s in range(1, S):
        nxt = (alpha @ Mx) * np.exp(em[:, s, :])
        m = mask[:, s].astype(bool)
        alpha = np.where(m[:, None], nxt, alpha)
        c = alpha.sum(axis=1)
        alpha /= c[:, None]
        logz += np.log(c)
    final = alpha * np.exp(end_transitions.astype(np.float64))[None, :]
    return logz + np.log(final.sum(axis=1))


def _run_device(emissions, start_transitions, end_transitions, transitions,
                trace=False):
    from concourse.bass_utils import run_bass_kernel_spmd

    if "nc" not in _CACHE:
        _CACHE["nc"] = _build_nc()
    nc = _CACHE["nc"]

    expM = np.exp(transitions.astype(np.float32))
    startT = start_transitions.astype(np.float32).reshape(T, 1)
    expEnd = np.exp(end_transitions.astype(np.float32)).reshape(T, 1)
    NR = S // RENORM
    in_maps = []
    for c in range(NCORES):
        in_maps.append({
            "em": np.ascontiguousarray(
                emissions[c * BL : (c + 1) * BL]).astype(np.float32).reshape(BL, S * T),
            "expM": expM,
            "startT": startT,
            "expEnd": expEnd,
        })
    res = run_bass_kernel_spmd(nc, in_maps, list(range(NCORES)), trace=trace)
    denoms = []
    for c in range(NCORES):
        strip = res.results[c]["cs"].reshape(NR + 1, BL).astype(np.float64)
        denoms.append(np.log(strip).sum(axis=0))
    return np.concatenate(denoms), res


def kernel(emissions, tags, mask, start_transitions, end_transitions, transitions):
    emissions = np.asarray(emissions, dtype=np.float32)
    tags = np.asarray(tags)
    mask = np.asarray(mask)
    start_transitions = np.asarray(start_transitions, dtype=np.float32)
    end_transitions = np.asarray(end_transitions, dtype=np.float32)
    transitions = np.asarray(transitions, dtype=np.float32)

    score = _numerator(emissions, tags, mask, start_transitions,
                       end_transitions, transitions)

    if np.all(mask != 0):
        denom, _ = _run_device(emissions, start_transitions, end_transitions,
                               transitions)
    else:
        denom = _denominator_host(emissions, mask, start_transitions,
                                  end_transitions, transitions)

    llh = denom.astype(np.float64) - score
    return np.float32(np.mean(llh))



# revision 3
# speedup vs baseline: 1.1657x; 1.1657x over previous
"""CRF NLL (mean) loss kernel for Trainium2, 8 NeuronCores.

Strategy (hardcoded for B=256, S=512, T=64):
  - Data-parallel over batch: 32 sequences per core.
  - Denominator (log-partition) on device via a BIDIRECTIONAL exp-space scan:
    forward chain from s=0 and backward chain from s=511 run fused as one
    [128, 32] state (top 64 partitions = fwd alpha^T, bottom = bwd beta^T),
    meeting in the middle after 255 rows:
        rhs_{j+1} = (WD.T @ rhs_j) * E_j
    with WD = blockdiag(expM, expM.T) bf16 stationary, E_j the stacked
    transposed emission exponentials exp(em - CBAR) in bf16.  The CBAR
    prescale keeps values in f32/bf16 range with NO renormalization
    (validated: max denom err 0.04 nats vs f64 at |denom|~2400).
  - Emissions are packed on host as fwd half [s=0..255] and REVERSED bwd
    half [s=511..256] so both chains read ascending; ACT exps them into
    paired 128-wide blocks; the DMA xbar transposes each [32,128] block
    to a [128,32] E tile (dtype bf16 as the xbar requires).
  - Z_b = sum_t alpha_255[t,b] * beta_255[t,b] computed on host in f64 from
    the two [128,32] outputs (rhs_255 bf16, ps_255 f32).
  - Numerator (gold path score) on host in numpy (gathers; ~0.3% of FLOPs).
  - Final mean on host.
"""

import sys

import numpy as np
import ml_dtypes

sys.path.insert(0, "/opt/trn_rl_repo")

B, S, T = 256, 512, 64
NCORES = 8
BL = B // NCORES   # 32 sequences per core
HALF = S // 2      # 256 steps per chain direction
ROWS = HALF - 1    # 255 chain rows with an emission mul
CBAR = 4.7         # exp prescale: exp(em - CBAR); log Z += S*CBAR on host

_CACHE = {}


def _build_nc():
    import concourse.bass as bass
    import concourse.mybir as mybir
    from concourse import tile

    AF = mybir.ActivationFunctionType
    f32 = mybir.dt.float32
    bf16 = mybir.dt.bfloat16

    nc = bass.Bass()
    emF_d = nc.dram_tensor("emF", [BL, HALF * T], f32, kind="ExternalInput")
    emB_d = nc.dram_tensor("emB", [BL, HALF * T], f32, kind="ExternalInput")
    wd_d = nc.dram_tensor("wd", [2 * T, 2 * T], bf16, kind="ExternalInput")
    scol_d = nc.dram_tensor("scol", [2 * T, 1], f32, kind="ExternalInput")
    orhs_d = nc.dram_tensor("orhs", [2 * T, BL], bf16, kind="ExternalOutput")
    ops_d = nc.dram_tensor("ops", [2 * T, BL], f32, kind="ExternalOutput")

    # pair tile c holds row-blocks: c=0 -> rows 0..62, c>=1 -> rows
    # 64c-1..64c+62; block [j][0:64] = exp(emF step j+1), [64:128] = exp(emB
    # step j+1).  A separate init tile holds [exp(emF 0) | exp(emB 0)].
    with tile.TileContext(nc) as tc:
        with (
            tc.tile_pool(name="consts", bufs=1) as consts,
            tc.tile_pool(name="emc", bufs=3) as emp,
            tc.tile_pool(name="pair", bufs=1) as pairp,
            tc.tile_pool(name="et", bufs=24) as etp,
            tc.tile_pool(name="rhs", bufs=4) as rp,
            tc.tile_pool(name="fin", bufs=1) as finp,
            tc.tile_pool(name="psum", bufs=4, space="PSUM") as psp,
        ):
            wd = consts.tile([2 * T, 2 * T], bf16)
            scol = consts.tile([2 * T, 1], f32)
            nbias = consts.tile([BL, 1], f32)
            nc.sync.dma_start(wd[:], wd_d[:])
            nc.sync.dma_start(scol[:], scol_d[:])
            nc.vector.memset(nbias[:], -CBAR)

            pair0 = pairp.tile([BL, 63, 2 * T], bf16, tag="pair0")
            pairs = [pair0] + [
                pairp.tile([BL, 64, 2 * T], bf16, tag=f"pair{c}",
                           name=f"pair{c}")
                for c in (1, 2, 3)
            ]
            pinit = pairp.tile([BL, 2 * T], bf16, tag="pinit")

            # DMA emission chunks + exp into pair blocks; fwd/bwd interleaved
            # so both chain heads are fed first.
            for c in range(4):
                for which, src in (("F", emF_d), ("B", emB_d)):
                    ch = emp.tile([BL, 64 * T], f32, tag="emc")
                    nc.sync.dma_start(ch[:], src[:, c * 64 * T:(c + 1) * 64 * T])
                    chv = ch[:].rearrange("p (s t) -> p s t", t=T)
                    lo, hi = (0, T) if which == "F" else (T, 2 * T)
                    if c == 0:
                        nc.scalar.activation(pinit[:, lo:hi], chv[:, 0, :],
                                             AF.Exp, bias=nbias[:])
                        nc.scalar.activation(pairs[0][:, :, lo:hi],
                                             chv[:, 1:64, :], AF.Exp,
                                             bias=nbias[:])
                    else:
                        # steps 64c..64c+63 -> rows 64c-1..64c+62 = tile c
                        nc.scalar.activation(pairs[c][:, :, lo:hi],
                                             chv[:], AF.Exp, bias=nbias[:])

            # init state: rhs_0 = E_init * [exp(start); exp(end)]
            einit = etp.tile([2 * T, BL], bf16, tag="et")
            nc.sync.dma_start(einit[:], pinit[:], transpose=True)
            rhs = rp.tile([2 * T, BL], bf16, tag="rhs")
            nc.vector.tensor_scalar_mul(rhs[:], einit[:], scol[:])

            for j in range(ROWS):
                c = 0 if j <= 62 else (j + 1) // 64
                blk = j if c == 0 else j - (64 * c - 1)
                et = etp.tile([2 * T, BL], bf16, tag="et")
                nc.sync.dma_start(et[:], pairs[c][:, blk, :], transpose=True)
                ps = psp.tile([2 * T, BL], f32, tag="ps")
                nc.tensor.matmul(ps[:], wd[:], rhs[:])
                rhs2 = rp.tile([2 * T, BL], bf16, tag="rhs")
                nc.vector.tensor_mul(rhs2[:], ps[:], et[:])
                rhs = rhs2

            # final matmul row (no emission mul); outputs to host
            ps = psp.tile([2 * T, BL], f32, tag="ps")
            nc.tensor.matmul(ps[:], wd[:], rhs[:])
            fin = finp.tile([2 * T, BL], f32)
            nc.scalar.copy(fin[:], ps[:])
            nc.sync.dma_start(orhs_d[:], rhs[:])
            nc.sync.dma_start(ops_d[:], fin[:])

    _split_multi_waits(nc)
    return nc


def _split_multi_waits(nc):
    # This toolchain's walrus rejects >1 sync-wait command per instruction
    # ("Too many sync wait commands").  Hoist all but the last wait of any
    # multi-wait instruction onto same-engine NoOps inserted just before it.
    import concourse.mybir as mybir

    for f in nc.m.functions:
        for bb in f.blocks:
            il = bb.instructions
            i = 0
            while i < len(il):
                inst = il[i]
                si = getattr(inst, "sync_info", None)
                if si is not None and len(si.on_wait) > 1:
                    waits = list(si.on_wait)
                    for k, w in enumerate(waits[:-1]):
                        nop = mybir.InstNoOp(
                            name=f"{inst.name}-w{k}", ins=[], outs=[])
                        nop.engine = inst.engine
                        nop.sync_info = mybir.SyncInfo(
                            on_wait=[w], on_update=[])
                        il.insert(i, nop)
                        i += 1
                    inst.sync_info = mybir.SyncInfo(
                        on_wait=[waits[-1]], on_update=list(si.on_update))
                i += 1


def _numerator(emissions, tags, mask, start_transitions, end_transitions, transitions):
    # Gold-path score per sequence, f64 accumulation on host.
    tg = tags.astype(np.int64)
    em = emissions.astype(np.float64)
    maskf = mask.astype(np.float64)
    b_idx = np.arange(B)
    emit = np.take_along_axis(em, tg[:, :, None], axis=2)[..., 0]      # [B, S]
    trans_sc = transitions.astype(np.float64)[tg[:, :-1], tg[:, 1:]]   # [B, S-1]
    score = start_transitions.astype(np.float64)[tg[:, 0]] + emit[:, 0]
    score = score + np.sum((trans_sc + emit[:, 1:]) * maskf[:, 1:], axis=1)
    seq_ends = np.sum(mask != 0, axis=1).astype(np.int64) - 1
    last_tags = tg[b_idx, seq_ends]
    score = score + end_transitions.astype(np.float64)[last_tags]
    return score  # [B] f64


def _denominator_host(emissions, mask, start_transitions, end_transitions, transitions):
    # General-mask fallback (never hit for the spec'd all-ones mask): scaled
    # exp-space forward scan in f64 on host.
    em = emissions.astype(np.float64)
    Mx = np.exp(transitions.astype(np.float64))
    alpha = np.exp(start_transitions.astype(np.float64)[None, :] + em[:, 0, :])
    logz = np.zeros(B)
    for s in range(1, S):
        nxt = (alpha @ Mx) * np.exp(em[:, s, :])
        m = mask[:, s].astype(bool)
        alpha = np.where(m[:, None], nxt, alpha)
        c = alpha.sum(axis=1)
        alpha /= c[:, None]
        logz += np.log(c)
    final = alpha * np.exp(end_transitions.astype(np.float64))[None, :]
    return logz + np.log(final.sum(axis=1))


def _run_device(emissions, start_transitions, end_transitions, transitions,
                trace=False):
    from concourse.bass_utils import run_bass_kernel_spmd

    if "nc" not in _CACHE:
        _CACHE["nc"] = _build_nc()
    nc = _CACHE["nc"]

    expM = np.exp(transitions.astype(np.float64))
    wd = np.zeros((2 * T, 2 * T), dtype=np.float64)
    wd[0:T, 0:T] = expM
    wd[T:2 * T, T:2 * T] = expM.T
    wd = wd.astype(ml_dtypes.bfloat16)
    scol = np.concatenate([
        np.exp(start_transitions.astype(np.float64)),
        np.exp(end_transitions.astype(np.float64)),
    ]).reshape(2 * T, 1).astype(np.float32)

    em = np.asarray(emissions, dtype=np.float32)
    in_maps = []
    for c in range(NCORES):
        sh = em[c * BL:(c + 1) * BL]                       # [BL, S, T]
        emF = np.ascontiguousarray(sh[:, :HALF]).reshape(BL, HALF * T)
        emB = np.ascontiguousarray(sh[:, :HALF - 1:-1]).reshape(BL, HALF * T)
        in_maps.append({"emF": emF, "emB": emB, "wd": wd, "scol": scol})
    res = run_bass_kernel_spmd(nc, in_maps, list(range(NCORES)), trace=trace)

    denoms = []
    for c in range(NCORES):
        top = res.results[c]["orhs"][0:T, :].astype(np.float64)   # alpha_255
        bot = res.results[c]["ops"][T:2 * T, :].astype(np.float64)  # beta_255
        Z = (top * bot).sum(axis=0)                               # [BL]
        denoms.append(np.log(Z) + S * CBAR)
    return np.concatenate(denoms), res


def kernel(emissions, tags, mask, start_transitions, end_transitions, transitions):
    emissions = np.asarray(emissions, dtype=np.float32)
    tags = np.asarray(tags)
    mask = np.asarray(mask)
    start_transitions = np.asarray(start_transitions, dtype=np.float32)
    end_transitions = np.asarray(end_transitions, dtype=np.float32)
    transitions = np.asarray(transitions, dtype=np.float32)

    score = _numerator(emissions, tags, mask, start_transitions,
                       end_transitions, transitions)

    if np.all(mask != 0):
        denom, _ = _run_device(emissions, start_transitions, end_transitions,
                               transitions)
    else:
        denom = _denominator_host(emissions, mask, start_transitions,
                                  end_transitions, transitions)

    llh = denom.astype(np.float64) - score
    return np.float32(np.mean(llh))


# revision 5
# speedup vs baseline: 2.1635x; 1.8560x over previous
"""CRF NLL (mean) loss kernel for Trainium2, 8 NeuronCores.

Strategy (hardcoded for B=256, S=512, T=64):
  - Data-parallel over batch: 32 sequences per core.
  - Denominator (log-partition) on device via a BIDIRECTIONAL exp-space scan:
    forward chain from s=0 and backward chain from s=511 run fused as one
    [128, 32] state (top 64 partitions = fwd alpha^T, bottom = bwd beta^T),
    meeting in the middle after 255 rows:
        rhs_{j+1} = (WD.T @ rhs_j) * E_j
    with WD = blockdiag(expM, expM.T) bf16 stationary, E_j the stacked
    transposed emission exponentials exp(em - CBAR) in bf16.  The CBAR
    prescale keeps values in f32/bf16 range with NO renormalization
    (validated: max denom err 0.04 nats vs f64 at |denom|~2400).
  - Emissions are packed on host as fwd half [s=0..255] and REVERSED bwd
    half [s=511..256] so both chains read ascending; ACT exps them into
    paired 128-wide blocks; the DMA xbar transposes each [32,128] block
    to a [128,32] E tile (dtype bf16 as the xbar requires).
  - Z_b = sum_t alpha_255[t,b] * beta_255[t,b] computed on host in f64 from
    the two [128,32] outputs (rhs_255 bf16, ps_255 f32).
  - Numerator (gold path score) on host in numpy (gathers; ~0.3% of FLOPs).
  - Final mean on host.
"""

import sys

import numpy as np
import ml_dtypes

sys.path.insert(0, "/opt/trn_rl_repo")

B, S, T = 256, 512, 64
NCORES = 8
BL = B // NCORES   # 32 sequences per core
HALF = S // 2      # 256 steps per chain direction
ROWS = HALF - 1    # 255 chain rows with an emission mul
CBAR = 4.7         # exp prescale: exp(em - CBAR); log Z += S*CBAR on host

_CACHE = {}


def _build_nc():
    import concourse.bass as bass
    import concourse.mybir as mybir
    from concourse import tile

    AF = mybir.ActivationFunctionType
    f32 = mybir.dt.float32
    bf16 = mybir.dt.bfloat16

    nc = bass.Bass()
    emF_d = nc.dram_tensor("emF", [BL, HALF * T], f32, kind="ExternalInput")
    emB_d = nc.dram_tensor("emB", [BL, HALF * T], f32, kind="ExternalInput")
    wd_d = nc.dram_tensor("wd", [2 * T, 2 * T], bf16, kind="ExternalInput")
    scol_d = nc.dram_tensor("scol", [2 * T, 1], f32, kind="ExternalInput")
    orhs_d = nc.dram_tensor("orhs", [2 * T, BL], bf16, kind="ExternalOutput")
    ops_d = nc.dram_tensor("ops", [2 * T, BL], f32, kind="ExternalOutput")

    # pair tile c holds row-blocks: c=0 -> rows 0..62, c>=1 -> rows
    # 64c-1..64c+62; block [j][0:64] = exp(emF step j+1), [64:128] = exp(emB
    # step j+1).  A separate init tile holds [exp(emF 0) | exp(emB 0)].
    with tile.TileContext(nc) as tc:
        with (
            tc.tile_pool(name="consts", bufs=1) as consts,
            tc.tile_pool(name="emc", bufs=3) as emp,
            tc.tile_pool(name="pair", bufs=1) as pairp,
            tc.tile_pool(name="et", bufs=1) as etp,
            tc.tile_pool(name="rhs", bufs=4) as rp,
            tc.tile_pool(name="fin", bufs=1) as finp,
            tc.tile_pool(name="psum", bufs=4, space="PSUM") as psp,
        ):
            wd = consts.tile([2 * T, 2 * T], bf16)
            scol = consts.tile([2 * T, 1], f32)
            nbias = consts.tile([BL, 1], f32)
            nc.sync.dma_start(wd[:], wd_d[:])
            nc.sync.dma_start(scol[:], scol_d[:])
            nc.vector.memset(nbias[:], -CBAR)

            pair0 = pairp.tile([BL, 63, 2 * T], bf16, tag="pair0")
            pairs = [pair0] + [
                pairp.tile([BL, 64, 2 * T], bf16, tag=f"pair{c}",
                           name=f"pair{c}")
                for c in (1, 2, 3)
            ]
            pinit = pairp.tile([BL, 2 * T], bf16, tag="pinit")

            # DMA emission chunks + exp into pair blocks; fwd/bwd interleaved
            # so both chain heads are fed first.
            for c in range(4):
                for which, src in (("F", emF_d), ("B", emB_d)):
                    ch = emp.tile([BL, 64 * T], f32, tag="emc")
                    nc.sync.dma_start(ch[:], src[:, c * 64 * T:(c + 1) * 64 * T])
                    chv = ch[:].rearrange("p (s t) -> p s t", t=T)
                    lo, hi = (0, T) if which == "F" else (T, 2 * T)
                    if c == 0:
                        nc.scalar.activation(pinit[:, lo:hi], chv[:, 0, :],
                                             AF.Exp, bias=nbias[:])
                        nc.scalar.activation(pairs[0][:, :, lo:hi],
                                             chv[:, 1:64, :], AF.Exp,
                                             bias=nbias[:])
                    else:
                        # steps 64c..64c+63 -> rows 64c-1..64c+62 = tile c
                        nc.scalar.activation(pairs[c][:, :, lo:hi],
                                             chv[:], AF.Exp, bias=nbias[:])

            # Blocked xbar transposes: ONE DMA per pair tile turns
            # [32, nblk*128] into [128, nblk, 32] (per-128-col-block
            # transpose), i.e. all E tiles of that chunk at once.
            ets = []
            for c in range(4):
                nblk = 63 if c == 0 else 64
                ett = etp.tile([2 * T, nblk, BL], bf16, tag=f"et{c}",
                               name=f"et{c}")
                nc.sync.dma_start(ett[:], pairs[c][:], transpose=True)
                ets.append(ett)

            # init state: rhs_0 = E_init * [exp(start); exp(end)]
            einit = etp.tile([2 * T, BL], bf16, tag="einit")
            nc.sync.dma_start(einit[:], pinit[:], transpose=True)
            rhs = rp.tile([2 * T, BL], bf16, tag="rhs")
            nc.vector.tensor_scalar_mul(rhs[:], einit[:], scol[:])

            for j in range(ROWS):
                c = 0 if j <= 62 else (j + 1) // 64
                blk = j if c == 0 else j - (64 * c - 1)
                ps = psp.tile([2 * T, BL], f32, tag="ps")
                nc.tensor.matmul(ps[:], wd[:], rhs[:])
                rhs2 = rp.tile([2 * T, BL], bf16, tag="rhs")
                nc.vector.tensor_mul(rhs2[:], ps[:], ets[c][:, blk, :])
                rhs = rhs2

            # final matmul row (no emission mul); outputs to host
            ps = psp.tile([2 * T, BL], f32, tag="ps")
            nc.tensor.matmul(ps[:], wd[:], rhs[:])
            fin = finp.tile([2 * T, BL], f32)
            nc.scalar.copy(fin[:], ps[:])
            nc.sync.dma_start(orhs_d[:], rhs[:])
            nc.sync.dma_start(ops_d[:], fin[:])

    _split_multi_waits(nc)
    return nc


def _split_multi_waits(nc):
    # This toolchain's walrus rejects >1 sync-wait command per instruction
    # ("Too many sync wait commands").  Hoist all but the last wait of any
    # multi-wait instruction onto same-engine NoOps inserted just before it.
    import concourse.mybir as mybir

    for f in nc.m.functions:
        for bb in f.blocks:
            il = bb.instructions
            i = 0
            while i < len(il):
                inst = il[i]
                si = getattr(inst, "sync_info", None)
                if si is not None and len(si.on_wait) > 1:
                    waits = list(si.on_wait)
                    for k, w in enumerate(waits[:-1]):
                        nop = mybir.InstNoOp(
                            name=f"{inst.name}-w{k}", ins=[], outs=[])
                        nop.engine = inst.engine
                        nop.sync_info = mybir.SyncInfo(
                            on_wait=[w], on_update=[])
                        il.insert(i, nop)
                        i += 1
                    inst.sync_info = mybir.SyncInfo(
                        on_wait=[waits[-1]], on_update=list(si.on_update))
                i += 1


def _numerator(emissions, tags, mask, start_transitions, end_transitions, transitions):
    # Gold-path score per sequence, f64 accumulation on host.
    tg = tags.astype(np.int64)
    em = emissions.astype(np.float64)
    maskf = mask.astype(np.float64)
    b_idx = np.arange(B)
    emit = np.take_along_axis(em, tg[:, :, None], axis=2)[..., 0]      # [B, S]
    trans_sc = transitions.astype(np.float64)[tg[:, :-1], tg[:, 1:]]   # [B, S-1]
    score = start_transitions.astype(np.float64)[tg[:, 0]] + emit[:, 0]
    score = score + np.sum((trans_sc + emit[:, 1:]) * maskf[:, 1:], axis=1)
    seq_ends = np.sum(mask != 0, axis=1).astype(np.int64) - 1
    last_tags = tg[b_idx, seq_ends]
    score = score + end_transitions.astype(np.float64)[last_tags]
    return score  # [B] f64


def _denominator_host(emissions, mask, start_transitions, end_transitions, transitions):
    # General-mask fallback (never hit for the spec'd all-ones mask): scaled
    # exp-space forward scan in f64 on host.
    em = emissions.astype(np.float64)
    Mx = np.exp(transitions.astype(np.float64))
    alpha = np.exp(start_transitions.astype(np.float64)[None, :] + em[:, 0, :])
    logz = np.zeros(B)
    for s in range(1, S):
        nxt = (alpha @ Mx) * np.exp(em[:, s, :])
        m = mask[:, s].astype(bool)
        alpha = np.where(m[:, None], nxt, alpha)
        c = alpha.sum(axis=1)
        alpha /= c[:, None]
        logz += np.log(c)
    final = alpha * np.exp(end_transitions.astype(np.float64))[None, :]
    return logz + np.log(final.sum(axis=1))


def _run_device(emissions, start_transitions, end_transitions, transitions,
                trace=False):
    from concourse.bass_utils import run_bass_kernel_spmd

    if "nc" not in _CACHE:
        _CACHE["nc"] = _build_nc()
    nc = _CACHE["nc"]

    expM = np.exp(transitions.astype(np.float64))
    wd = np.zeros((2 * T, 2 * T), dtype=np.float64)
    wd[0:T, 0:T] = expM
    wd[T:2 * T, T:2 * T] = expM.T
    wd = wd.astype(ml_dtypes.bfloat16)
    scol = np.concatenate([
        np.exp(start_transitions.astype(np.float64)),
        np.exp(end_transitions.astype(np.float64)),
    ]).reshape(2 * T, 1).astype(np.float32)

    em = np.asarray(emissions, dtype=np.float32)
    in_maps = []
    for c in range(NCORES):
        sh = em[c * BL:(c + 1) * BL]                       # [BL, S, T]
        emF = np.ascontiguousarray(sh[:, :HALF]).reshape(BL, HALF * T)
        emB = np.ascontiguousarray(sh[:, :HALF - 1:-1]).reshape(BL, HALF * T)
        in_maps.append({"emF": emF, "emB": emB, "wd": wd, "scol": scol})
    res = run_bass_kernel_spmd(nc, in_maps, list(range(NCORES)), trace=trace)

    denoms = []
    for c in range(NCORES):
        top = res.results[c]["orhs"][0:T, :].astype(np.float64)   # alpha_255
        bot = res.results[c]["ops"][T:2 * T, :].astype(np.float64)  # beta_255
        Z = (top * bot).sum(axis=0)                               # [BL]
        denoms.append(np.log(Z) + S * CBAR)
    return np.concatenate(denoms), res


def kernel(emissions, tags, mask, start_transitions, end_transitions, transitions):
    emissions = np.asarray(emissions, dtype=np.float32)
    tags = np.asarray(tags)
    mask = np.asarray(mask)
    start_transitions = np.asarray(start_transitions, dtype=np.float32)
    end_transitions = np.asarray(end_transitions, dtype=np.float32)
    transitions = np.asarray(transitions, dtype=np.float32)

    score = _numerator(emissions, tags, mask, start_transitions,
                       end_transitions, transitions)

    if np.all(mask != 0):
        denom, _ = _run_device(emissions, start_transitions, end_transitions,
                               transitions)
    else:
        denom = _denominator_host(emissions, mask, start_transitions,
                                  end_transitions, transitions)

    llh = denom.astype(np.float64) - score
    return np.float32(np.mean(llh))
